# revision 5
# baseline (speedup 1.0000x reference)
"""AttentionDecoder step on 8 TRN2 NeuronCores (Bass/Tile).

Sharding: vocab-parallel out-projection (V padded 50257->51200, 6400/core),
feature-parallel small chain (attention-applied / comb / GRU sharded over the
H=1024 feature dim, 128/core), log_softmax via local sum-exp + AllGather.
Matmuls run in bf16 (fp32 accumulate in PSUM), elementwise math in fp32.
"""

import sys

import numpy as np

if "/opt/trn_rl_repo" not in sys.path:
    sys.path.insert(0, "/opt/trn_rl_repo")

import ml_dtypes  # noqa: E402

from concourse import bacc, bass, masks, mybir, tile  # noqa: E402
from concourse.bass_utils import run_bass_kernel_spmd  # noqa: E402

V, H, B, L, NC = 50257, 1024, 16, 100, 8
VPAD = 51200          # 8 * 6400
VS = VPAD // NC       # 6400 per core
VT = [512] * 12 + [256]   # v-tile widths per core (sum = 6400)
KC = H // 128         # 8 k-chunks of 128 over H
HS = H // NC          # 128: per-core feature slice of the small chain
GS = 3 * HS           # 384: per-core GRU gate slice (r|z|n)
NEG = -60.0           # pad bias: exp(-60)~9e-27 vanishes in the sum, Ln stays finite

BF = mybir.dt.bfloat16
F32 = mybir.dt.float32
F32R = mybir.dt.float32r
ACTF = mybir.ActivationFunctionType
AX = mybir.AxisListType

BF_NP = ml_dtypes.bfloat16


def build_graph(n_wbufs: int = 10):
    nc = bacc.Bacc(
        "TRN2", target_bir_lowering=False, debug=False, num_devices=NC
    )

    # ---- kernel I/O (per-core shards; the graph itself is core-uniform) ----
    def inp(name, shape, dt):
        return nc.dram_tensor(name, shape, dt, kind="ExternalInput")

    d_embT = inp("embT", [H, B], BF)            # emb[inputs].T, replicated
    d_h0T = inp("h0T", [H, B], BF)              # hidden[0].T, replicated
    d_h0c = inp("h0c", [B, HS], F32)            # hidden[0][:, c*128:(c+1)*128]
    d_WaT = inp("WaT", [2 * H, L], BF)          # W_attn.T, replicated
    d_ba = inp("ba", [1, L], F32)               # b_attn row
    d_enc = inp("encc", [L, B * HS], BF)        # enc[:, :, c-slice] flattened
    d_WcT = inp("WcT", [2 * H, HS], BF)         # W_comb.T[:, c-slice]
    d_bc = inp("bc", [HS, 1], F32)              # b_comb[c-slice] column
    d_WiT = inp("WiT", [H, GS], BF)             # W_ih.T r|z|n c-slices
    d_WhT = inp("WhT", [H, GS], BF)             # W_hh.T r|z|n c-slices
    d_bi = inp("bi", [1, GS], BF)
    d_bh = inp("bh", [1, GS], BF)
    d_WoT = inp("WoT", [H, VS], BF)             # W_out.T vocab shard
    d_bo = inp("bo", [1, VS], BF)              # b_out shard (pad = NEG)
    d_ones = inp("ones", [1, B], F32)

    d_out = nc.dram_tensor("out", [B, VS], F32, kind="ExternalOutput")
    d_h1c = nc.dram_tensor("h1c", [B, HS], F32, kind="ExternalOutput")
    d_attnw = nc.dram_tensor("attnw", [B, L], F32, kind="ExternalOutput")

    rg = [list(range(NC))]

    with tile.TileContext(nc) as tc:
        with (
            tc.tile_pool(name="dram", bufs=1, space="DRAM") as dpool,
            tc.tile_pool(name="const", bufs=1) as cpool,
            tc.tile_pool(name="acts", bufs=1) as apool,
            tc.tile_pool(name="psA", bufs=2, space="PSUM") as psA,
            tc.tile_pool(name="psB", bufs=3, space="PSUM") as psB,
            tc.tile_pool(name="wS", bufs=n_wbufs) as wpool,
            tc.tile_pool(name="oS", bufs=3) as opool,
        ):
            # ---- persistent SBUF tiles + loads ----
            ident = cpool.tile([128, 128], BF, tag="ident")
            masks.make_identity(nc, ident[:, :])

            sb_catT = cpool.tile([128, 2 * KC, B], BF, tag="catT")
            nc.sync.dma_start(
                out=sb_catT[:, 0:KC, :],
                in_=d_embT.ap().rearrange("(k p) b -> p k b", p=128),
            )
            sb_h0T = cpool.tile([128, KC, B], BF, tag="h0T")
            nc.sync.dma_start(
                out=sb_h0T[:, :, :],
                in_=d_h0T.ap().rearrange("(k p) b -> p k b", p=128),
            )
            sb_h0c = cpool.tile([B, HS], F32, tag="h0c")
            nc.sync.dma_start(out=sb_h0c[:, :], in_=d_h0c.ap())
            sb_ones = cpool.tile([1, B], F32, tag="ones")
            nc.sync.dma_start(out=sb_ones[:, :], in_=d_ones.ap())
            sb_onesb = cpool.tile([1, B], BF, tag="onesb")
            nc.scalar.copy(sb_onesb[:, :], sb_ones[:, :])
            sb_ba = cpool.tile([1, L], F32, tag="ba")
            nc.sync.dma_start(out=sb_ba[:, :], in_=d_ba.ap())
            sb_bc = cpool.tile([HS, 1], F32, tag="bc")
            nc.sync.dma_start(out=sb_bc[:, :], in_=d_bc.ap())
            sb_bi = cpool.tile([1, GS], BF, tag="bi")
            nc.sync.dma_start(out=sb_bi[:, :], in_=d_bi.ap())
            sb_bh = cpool.tile([1, GS], BF, tag="bh")
            nc.sync.dma_start(out=sb_bh[:, :], in_=d_bh.ap())
            sb_bo = cpool.tile([1, VS], BF, tag="bo")
            nc.sync.dma_start(out=sb_bo[:, :], in_=d_bo.ap())

            sb_WaT = apool.tile([128, 2 * KC, L], BF, tag="WaT")
            nc.sync.dma_start(
                out=sb_WaT[:, :, :],
                in_=d_WaT.ap().rearrange("(k p) l -> p k l", p=128),
            )
            sb_enc = apool.tile([L, B * HS], BF, tag="enc")
            nc.sync.dma_start(out=sb_enc[:, :], in_=d_enc.ap())
            sb_WcT = apool.tile([128, 2 * KC, HS], BF, tag="WcT")
            nc.sync.dma_start(
                out=sb_WcT[:, :, :],
                in_=d_WcT.ap().rearrange("(k p) m -> p k m", p=128),
            )
            sb_WiT = apool.tile([128, KC, GS], BF, tag="WiT")
            nc.sync.dma_start(
                out=sb_WiT[:, :, :],
                in_=d_WiT.ap().rearrange("(k p) g -> p k g", p=128),
            )
            sb_WhT = apool.tile([128, KC, GS], BF, tag="WhT")
            nc.sync.dma_start(
                out=sb_WhT[:, :, :],
                in_=d_WhT.ap().rearrange("(k p) g -> p k g", p=128),
            )

            # ---- A1: attention logits [B, L] + softmax ----
            ps_l = psA.tile([B, L], F32, tag="psA")
            for k in range(KC):
                nc.tensor.matmul(
                    ps_l[:, :], sb_catT[:, k, :], sb_WaT[:, k, :],
                    start=(k == 0), stop=False,
                )
            for k in range(KC):
                nc.tensor.matmul(
                    ps_l[:, :], sb_h0T[:, k, :], sb_WaT[:, KC + k, :],
                    start=False, stop=False,
                )
            nc.tensor.matmul(
                ps_l[:, :], sb_ones[:, :], sb_ba[:, :],
                start=False, stop=True,
            )

            negmx = apool.tile([B, 1], F32, tag="negmx")
            nc.vector.tensor_reduce(
                out=negmx[:, :], in_=ps_l[:, :], axis=AX.X,
                op=mybir.AluOpType.max, negate=True,
            )
            e_l = apool.tile([B, L], F32, tag="e_l")
            se = apool.tile([B, 1], F32, tag="se")
            nc.scalar.activation(
                e_l[:, :], ps_l[:, :], ACTF.Exp,
                bias=negmx[:, 0:1], accum_out=se[:, 0:1],
            )
            rse = apool.tile([B, 1], F32, tag="rse")
            nc.vector.reciprocal(rse[:, :], se[:, :])
            w_f = apool.tile([B, L], F32, tag="w_f")
            nc.vector.tensor_scalar_mul(w_f[:, :], e_l[:, :], rse[:, 0:1])
            nc.sync.dma_start(out=d_attnw.ap(), in_=w_f[:, :])
            w_b = apool.tile([B, L], BF, tag="w_b")
            nc.scalar.copy(w_b[:, :], w_f[:, :])

            # ---- A2: attention applied, this core's h-slice ----
            ps_wt = psA.tile([L, B], BF, tag="psA")
            nc.tensor.transpose(ps_wt[:, :], w_b[:, :], ident[:B, :B])
            sb_wT = apool.tile([L, B], BF, tag="sb_wT")
            nc.scalar.copy(sb_wT[:, :], ps_wt[:, :])

            # per-b matmuls straight into appliedT columns: lhsT = enc b-slice
            # [100,128] (stationary), rhs = wT column b -> psum [128,1]
            ps_apT = psA.tile([HS, B], F32, tag="psA")
            for b in range(B):
                nc.tensor.matmul(
                    ps_apT[:, b : b + 1],
                    sb_enc[:, b * HS : (b + 1) * HS],
                    sb_wT[:, b : b + 1],
                    start=True, stop=True,
                )
            sb_apT = apool.tile([HS, B], BF, tag="sb_apT")
            nc.scalar.copy(sb_apT[:, :], ps_apT[:, :])

            ag1_in = dpool.tile([HS, B], BF, tag="ag1_in")
            ag1_out = dpool.tile([H, B], BF, tag="ag1_out")
            nc.gpsimd.dma_start(out=ag1_in[:, :], in_=sb_apT[:, :])
            nc.gpsimd.collective_compute(
                "AllGather", mybir.AluOpType.bypass, replica_groups=rg,
                ins=[ag1_in.opt()], outs=[ag1_out.opt()],
            )
            nc.gpsimd.dma_start(
                out=sb_catT[:, KC : 2 * KC, :],
                in_=ag1_out[:, :].rearrange("(k p) b -> p k b", p=128),
            )

            # ---- A3: combine + relu -> xT slice, AllGather to full xT ----
            ps_x = psA.tile([HS, B], F32, tag="psA")
            for k in range(2 * KC):
                nc.tensor.matmul(
                    ps_x[:, :], sb_WcT[:, k, :], sb_catT[:, k, :],
                    start=(k == 0), stop=(k == 2 * KC - 1),
                )
            sb_xTc = apool.tile([HS, B], BF, tag="sb_xTc")
            nc.scalar.activation(
                sb_xTc[:, :], ps_x[:, :], ACTF.Relu, bias=sb_bc[:, 0:1]
            )

            ag2_in = dpool.tile([HS, B], BF, tag="ag2_in")
            ag2_out = dpool.tile([H, B], BF, tag="ag2_out")
            nc.gpsimd.dma_start(out=ag2_in[:, :], in_=sb_xTc[:, :])
            nc.gpsimd.collective_compute(
                "AllGather", mybir.AluOpType.bypass, replica_groups=rg,
                ins=[ag2_in.opt()], outs=[ag2_out.opt()],
            )
            sb_xT = apool.tile([128, KC, B], BF, tag="sb_xT")
            nc.gpsimd.dma_start(
                out=sb_xT[:, :, :],
                in_=ag2_out[:, :].rearrange("(k p) b -> p k b", p=128),
            )

            # ---- A4: GRU gate slices -> h1 slice, AllGather to h1T ----
            ps_gi = psA.tile([B, GS], F32, tag="psA")
            for k in range(KC):
                nc.tensor.matmul(
                    ps_gi[:, :], sb_xT[:, k, :], sb_WiT[:, k, :],
                    start=(k == 0), stop=False,
                )
            nc.tensor.matmul(
                ps_gi[:, :], sb_onesb[:, :], sb_bi[:, :],
                start=False, stop=True,
            )
            ps_gh = psA.tile([B, GS], F32, tag="psA")
            for k in range(KC):
                nc.tensor.matmul(
                    ps_gh[:, :], sb_h0T[:, k, :], sb_WhT[:, k, :],
                    start=(k == 0), stop=False,
                )
            nc.tensor.matmul(
                ps_gh[:, :], sb_onesb[:, :], sb_bh[:, :],
                start=False, stop=True,
            )

            sb_gi = apool.tile([B, GS], F32, tag="sb_gi")
            nc.scalar.copy(sb_gi[:, :], ps_gi[:, :])
            sb_gh = apool.tile([B, GS], F32, tag="sb_gh")
            nc.scalar.copy(sb_gh[:, :], ps_gh[:, :])
            t_rz = apool.tile([B, 2 * HS], F32, tag="t_rz")
            nc.vector.tensor_add(
                t_rz[:, :], sb_gi[:, 0 : 2 * HS], sb_gh[:, 0 : 2 * HS]
            )
            rz = apool.tile([B, 2 * HS], F32, tag="rz")
            nc.scalar.activation(rz[:, :], t_rz[:, :], ACTF.Sigmoid)
            rh = apool.tile([B, HS], F32, tag="rh")
            nc.vector.tensor_mul(
                rh[:, :], rz[:, 0:HS], sb_gh[:, 2 * HS : GS]
            )
            tn = apool.tile([B, HS], F32, tag="tn")
            nc.vector.tensor_add(tn[:, :], sb_gi[:, 2 * HS : GS], rh[:, :])
            n_t = apool.tile([B, HS], F32, tag="n_t")
            nc.scalar.activation(n_t[:, :], tn[:, :], ACTF.Tanh)
            d_t = apool.tile([B, HS], F32, tag="d_t")
            nc.vector.tensor_sub(d_t[:, :], sb_h0c[:, :], n_t[:, :])
            dz = apool.tile([B, HS], F32, tag="dz")
            nc.vector.tensor_mul(dz[:, :], rz[:, HS : 2 * HS], d_t[:, :])
            h1_f = apool.tile([B, HS], F32, tag="h1_f")
            nc.vector.tensor_add(h1_f[:, :], n_t[:, :], dz[:, :])
            nc.sync.dma_start(out=d_h1c.ap(), in_=h1_f[:, :])

            h1_b = apool.tile([B, HS], BF, tag="h1_b")
            nc.scalar.copy(h1_b[:, :], h1_f[:, :])
            ps_h1T = psA.tile([HS, B], BF, tag="psA")
            nc.tensor.transpose(ps_h1T[:, :], h1_b[:, :], ident[:B, :B])
            sb_h1Tc = apool.tile([HS, B], BF, tag="sb_h1Tc")
            nc.scalar.copy(sb_h1Tc[:, :], ps_h1T[:, :])

            ag3_in = dpool.tile([HS, B], BF, tag="ag3_in")
            ag3_out = dpool.tile([H, B], BF, tag="ag3_out")
            nc.gpsimd.dma_start(out=ag3_in[:, :], in_=sb_h1Tc[:, :])
            nc.gpsimd.collective_compute(
                "AllGather", mybir.AluOpType.bypass, replica_groups=rg,
                ins=[ag3_in.opt()], outs=[ag3_out.opt()],
            )
            sb_h1T = apool.tile([128, KC, B], BF, tag="sb_h1T")
            nc.gpsimd.dma_start(
                out=sb_h1T[:, :, :],
                in_=ag3_out[:, :].rearrange("(k p) b -> p k b", p=128),
            )

            # ---- B: vocab-shard logits, exp + running sums ----
            sb_exp = cpool.tile([B, VS], F32, tag="sb_exp")
            sums = cpool.tile([B, len(VT)], F32, tag="sums")
            v0 = 0
            for t, nt in enumerate(VT):
                wt = wpool.tile([128, KC, VT[0]], BF, tag="w")
                nc.sync.dma_start(
                    out=wt[:, :, :nt],
                    in_=d_WoT.ap()[:, v0 : v0 + nt].rearrange(
                        "(k p) n -> p k n", p=128
                    ),
                )
                ps_t = psB.tile([B, VT[0]], F32, tag="psB")
                for k in range(KC):
                    nc.tensor.matmul(
                        ps_t[:, :nt], sb_h1T[:, k, :], wt[:, k, :nt],
                        start=(k == 0), stop=False,
                    )
                nc.tensor.matmul(
                    ps_t[:, :nt], sb_onesb[:, :], sb_bo[:, v0 : v0 + nt],
                    start=False, stop=True,
                )
                nc.scalar.activation(
                    sb_exp[:, v0 : v0 + nt], ps_t[:, :nt], ACTF.Exp,
                    accum_out=sums[:, t : t + 1],
                )
                v0 += nt

            # ---- C: global sum via AllGather, out = Ln(exp / S) ----
            s_loc = apool.tile([B, 1], F32, tag="s_loc")
            nc.vector.tensor_reduce(
                out=s_loc[:, :], in_=sums[:, :], axis=AX.X,
                op=mybir.AluOpType.add,
            )
            ag4_in = dpool.tile([1, B], F32, tag="ag4_in")
            ag4_out = dpool.tile([NC, B], F32, tag="ag4_out")
            nc.gpsimd.dma_start(out=ag4_in[0, :], in_=s_loc[:, 0])
            nc.gpsimd.collective_compute(
                "AllGather", mybir.AluOpType.bypass, replica_groups=rg,
                ins=[ag4_in.opt()], outs=[ag4_out.opt()],
            )
            sb_sall = apool.tile([B, NC], F32, tag="sb_sall")
            nc.gpsimd.dma_start(
                out=sb_sall[:, :],
                in_=ag4_out[:, :].rearrange("r b -> b r"),
            )
            s_glob = apool.tile([B, 1], F32, tag="s_glob")
            nc.vector.tensor_reduce(
                out=s_glob[:, :], in_=sb_sall[:, :], axis=AX.X,
                op=mybir.AluOpType.add,
            )
            r_glob = apool.tile([B, 1], F32, tag="r_glob")
            nc.vector.reciprocal(r_glob[:, :], s_glob[:, :])

            v0 = 0
            for t, nt in enumerate(VT):
                o_t = opool.tile([B, VT[0]], F32, tag="o")
                nc.scalar.activation(
                    o_t[:, :nt], sb_exp[:, v0 : v0 + nt], ACTF.Ln,
                    scale=r_glob[:, 0:1],
                )
                nc.sync.dma_start(
                    out=d_out.ap()[:, v0 : v0 + nt], in_=o_t[:, :nt]
                )
                v0 += nt

    nc.compile()
    return nc


def make_in_maps(inputs, hidden, encoder_outputs, emb, W_attn, b_attn,
                 W_comb, b_comb, W_ih, W_hh, b_ih, b_hh, W_out, b_out):
    idx = np.asarray(inputs).astype(np.int64)
    h0 = np.asarray(hidden, np.float32)[0]                  # [B,H]
    enc = np.asarray(encoder_outputs, np.float32)           # [L,B,H]
    emb = np.asarray(emb, np.float32)
    W_attn = np.asarray(W_attn, np.float32)
    b_attn = np.asarray(b_attn, np.float32)
    W_comb = np.asarray(W_comb, np.float32)
    b_comb = np.asarray(b_comb, np.float32)
    W_ih = np.asarray(W_ih, np.float32)
    W_hh = np.asarray(W_hh, np.float32)
    b_ih = np.asarray(b_ih, np.float32)
    b_hh = np.asarray(b_hh, np.float32)
    W_out = np.asarray(W_out, np.float32)
    b_out = np.asarray(b_out, np.float32)

    def bf(x):
        return np.ascontiguousarray(x.astype(BF_NP))

    embT = bf(emb[idx].T)                                   # [H,B]
    h0T = bf(h0.T)
    WaT = bf(W_attn.T)                                      # [2H,L]
    ba = np.ascontiguousarray(b_attn[None, :])
    WiT_f = W_ih.T                                          # [H,3H]
    WhT_f = W_hh.T
    Wo_pad = np.zeros((VPAD, H), np.float32)
    Wo_pad[:V] = W_out
    bo_pad = np.full(VPAD, NEG, np.float32)
    bo_pad[:V] = b_out
    ones = np.ones((1, B), np.float32)

    in_maps = []
    for c in range(NC):
        hs = slice(c * HS, (c + 1) * HS)
        gcols = np.r_[c * HS : (c + 1) * HS,
                      H + c * HS : H + (c + 1) * HS,
                      2 * H + c * HS : 2 * H + (c + 1) * HS]
        in_maps.append({
            "embT": embT,
            "h0T": h0T,
            "h0c": np.ascontiguousarray(h0[:, hs]),
            "WaT": WaT,
            "ba": ba,
            "encc": bf(enc[:, :, hs].reshape(L, B * HS)),
            "WcT": bf(W_comb.T[:, hs]),
            "bc": np.ascontiguousarray(b_comb[hs][:, None]),
            "WiT": bf(WiT_f[:, gcols]),
            "WhT": bf(WhT_f[:, gcols]),
            "bi": bf(b_ih[gcols][None, :]),
            "bh": bf(b_hh[gcols][None, :]),
            "WoT": bf(Wo_pad[c * VS : (c + 1) * VS].T),
            "bo": bf(bo_pad[None, c * VS : (c + 1) * VS]),
            "ones": ones,
        })
    return in_maps


_CACHE = {}


def _get_graph():
    if "nc" not in _CACHE:
        _CACHE["nc"] = build_graph()
    return _CACHE["nc"]


def run(in_maps, trace=False, **kw):
    nc = _get_graph()
    return run_bass_kernel_spmd(
        nc, in_maps, core_ids=list(range(NC)), trace=trace, **kw
    )


def kernel(**inputs):
    in_maps = make_in_maps(**inputs)
    res = run(in_maps)
    results = res.results
    out = np.concatenate([results[c]["out"] for c in range(NC)], axis=1)
    out = np.ascontiguousarray(out[:, :V])
    h1 = np.concatenate([results[c]["h1c"] for c in range(NC)], axis=1)
    attnw = results[0]["attnw"]
    return out, h1[None], attnw
